# revision 1
# baseline (speedup 1.0000x reference)
"""Causal self-attention Trainium2 kernel.

Full inputs -> full outputs. Data-parallel over batch across 8 NeuronCores
(16 batches per core), no collectives.

Per-core design (bf16 matmul operands, fp32 PSUM accumulate):
  - X [tok, C] fp32 is PE-transposed to XT bf16 [C, tok].
  - Q^T/K^T [feature, tok] bf16: lhsT = w_attn^T chunk, rhs = XT; the 1/8
    score scale is folded into the Q weights/bias on the host; bias applied
    on the ACT eviction.
  - V [tok, feature] bf16 with an interleaved ones column per head so row 64
    of the PV output is Z = sum_k P.
  - Scores per (batch, head) are computed transposed and PACKED [k=128, 384]:
    cols 0:256 = k-tile0 x (q 0..255), cols 256:384 = k-tile1 x (q 128..255);
    the fully-masked (k-tile1, q<128) quadrant is never computed. The causal
    mask (-1e30) is pre-added into PSUM via a bf16 identity matmul, then one
    ACT exp eviction -> P bf16.
  - PV: 3 N=128 matmuls -> O~ [65, 256] (row 64 = Z).
  - Normalization without any DRAM round-trip: a K=1 matmul broadcasts Z
    across 64 partitions into PSUM, then one DVE tensor_tensor divide writes
    O^T = O~/Z straight into the pair-stacked OT tile (head parity picks
    partitions 0:64 / 64:128), so projection runs with K=128 lhsT tiles.
  - Projection: bias pre-added via a K=1 rank-1 matmul (ones x beff_row),
    then 3 K=128 accumulating matmuls per 128-token tile; plain Pool-engine
    eviction, one batched output DMA per group.
"""

import numpy as np
import ml_dtypes

import concourse.bass as bass
import concourse.bacc as bacc
import concourse.mybir as mybir
import concourse.tile as tile

N_CORES = 8
B, T, C = 128, 256, 384
H, HD = 6, 64
NB = B // N_CORES          # batches per core
TOK = NB * T               # tokens per core
G = 2                      # batches per group
NG = NB // G               # groups per core
GT = G * T                 # tokens per group (512)
NTT = GT // 128            # 128-token tiles per group (4)
F32 = mybir.dt.float32
F32R = mybir.dt.float32r
BF16 = mybir.dt.bfloat16
AF = mybir.ActivationFunctionType
ALU = mybir.AluOpType
NEGBIG = -1.0e30


def _body(tc, x_d, wat_d, wpt_d, bq_d, bk_d, beff_d, trimask_d,
          identb_d, y_d):
    nc = tc.nc
    from contextlib import ExitStack

    ctx = ExitStack()
    with ctx:
        const = ctx.enter_context(tc.tile_pool(name="const", bufs=1))
        xin = ctx.enter_context(tc.tile_pool(name="xin", bufs=2))
        xt = ctx.enter_context(tc.tile_pool(name="xt", bufs=2))
        qkt = ctx.enter_context(tc.tile_pool(name="qkt", bufs=2))
        v65 = ctx.enter_context(tc.tile_pool(name="v65", bufs=2))
        pp = ctx.enter_context(tc.tile_pool(name="pp", bufs=6))
        oe = ctx.enter_context(tc.tile_pool(name="oe", bufs=6))
        on = ctx.enter_context(tc.tile_pool(name="on", bufs=4))
        ot = ctx.enter_context(tc.tile_pool(name="ot", bufs=2))
        yb = ctx.enter_context(tc.tile_pool(name="yb", bufs=2))
        mm_ps = ctx.enter_context(tc.tile_pool(name="mm_ps", bufs=2, space="PSUM"))
        s_ps = ctx.enter_context(tc.tile_pool(name="s_ps", bufs=2, space="PSUM"))
        o_ps = ctx.enter_context(tc.tile_pool(name="o_ps", bufs=2, space="PSUM"))
        tr_ps = ctx.enter_context(tc.tile_pool(name="tr_ps", bufs=2, space="PSUM"))

        dma = nc.sync.dma_start

        wat_sb = const.tile([128, 3, 3 * C], BF16, name="wat_sb")
        wpt_sb = const.tile([128, 3, C], BF16, name="wpt_sb")
        bq_sb = const.tile([128, 3], F32, name="bq_sb")
        bk_sb = const.tile([128, 3], F32, name="bk_sb")
        beff_sb = const.tile([128, C], F32, name="beff_sb")
        trimask_sb = const.tile([128, 128], BF16, name="trimask_sb")
        identb_sb = const.tile([128, 128], BF16, name="identb_sb")

        dma(wat_sb[:], wat_d.ap().rearrange("(ct p) f -> p ct f", p=128))
        dma(wpt_sb[:], wpt_d.ap())
        dma(bq_sb[:], bq_d.ap())
        dma(bk_sb[:], bk_d.ap())
        dma(beff_sb[:], beff_d.ap())
        dma(trimask_sb[:], trimask_d.ap())
        dma(identb_sb[:], identb_d.ap())

        xv = x_d.ap().rearrange("(g tt p) c -> g p tt c", tt=NTT, p=128)
        yv = y_d.ap().rearrange("(g tt p) c -> g p tt c", tt=NTT, p=128)

        # Tiles live across the gen(g+1) / attention(g) software pipeline.
        st = {}

        def gen_chunks(g):
            """QKV generation for group g as a list of emit-closures."""
            X_sb = xin.tile([128, NTT, C], BF16, name=f"X_{g}", tag="X")
            XT_sb = xt.tile([128, 3, GT], BF16, name=f"XT_{g}", tag="XT")
            QKT_sb = qkt.tile([128, 6, GT], BF16, name=f"QKT_{g}", tag="QKT")
            V65_sb = v65.tile([128, NTT, H * 65], BF16, name=f"V65_{g}",
                              tag="V65")
            st[g] = (QKT_sb, V65_sb)
            chunks = [lambda: dma(X_sb[:], xv[g])]

            def trans(ct):
                ps_t = mm_ps.tile([128, 512], BF16, name=f"pst_{g}_{ct}",
                                  tag="mm")
                for tt in range(NTT):
                    nc.tensor.transpose(
                        ps_t[:, 128 * tt:128 * (tt + 1)],
                        X_sb[:, tt, 128 * ct:128 * (ct + 1)],
                        identb_sb[:],
                    )
                nc.vector.tensor_copy(XT_sb[:, ct, :], ps_t[:])

            def qkgen(ft):
                ps_qk = mm_ps.tile([128, 512], F32, name=f"psqk_{g}_{ft}",
                                   tag="mm")
                for ct in range(3):
                    nc.tensor.matmul(
                        ps_qk[:],
                        wat_sb[:, ct, 128 * ft:128 * (ft + 1)],
                        XT_sb[:, ct, :],
                        start=(ct == 0),
                        stop=(ct == 2),
                    )
                bias = bq_sb[:, ft:ft + 1] if ft < 3 else bk_sb[:, ft - 3:ft - 2]
                nc.scalar.activation(QKT_sb[:, ft, :], ps_qk[:], AF.Identity,
                                     bias=bias)

            def vgen(tt):
                ps_v = mm_ps.tile([128, 384], F32, name=f"psv_{g}_{tt}",
                                  tag="mm")
                for ct in range(3):
                    nc.tensor.matmul(
                        ps_v[:],
                        XT_sb[:, ct, 128 * tt:128 * (tt + 1)],
                        wat_sb[:, ct, 2 * C:3 * C],
                        start=(ct == 0),
                        stop=(ct == 2),
                    )
                v_view = V65_sb[:, tt, :].rearrange("p (h w) -> p h w", h=H)
                if tt % 2 == 0:
                    nc.vector.tensor_copy(
                        v_view[:, :, 0:64],
                        ps_v[:].rearrange("p (h w) -> p h w", h=H),
                    )
                else:
                    nc.scalar.copy(
                        v_view[:, :, 0:64],
                        ps_v[:].rearrange("p (h w) -> p h w", h=H),
                    )
                nc.gpsimd.memset(v_view[:, :, 64:65], 1.0)

            from functools import partial
            chunks += [partial(trans, ct) for ct in range(3)]
            chunks += [partial(qkgen, ft) for ft in range(6)]
            chunks += [partial(vgen, tt) for tt in range(NTT)]
            return chunks

        def att_chunks(g):
            """Attention + projection for group g as a list of emit-closures.

            The per-head work is stage-split (scores -> PV -> normalize) and
            emitted in waves [scores(i), pv(i-1), norm(i-2)] so the PE never
            sits directly behind an ACT exp or an Oe eviction.
            """
            QKT_sb, V65_sb = st[g]
            OT_sb = ot.tile([128, 3, GT], BF16, name=f"OT_{g}", tag="OT")
            Y_sb = yb.tile([128, NTT, C], F32, name=f"Y_{g}", tag="Y")
            hs = {}

            def scores(bl, h):
                q0 = 256 * bl
                ft, r0 = h // 2, 64 * (h % 2)
                KT = QKT_sb[r0:r0 + 64, 3 + ft, :]
                QT = QKT_sb[r0:r0 + 64, ft, :]

                ps_s = s_ps.tile([128, 384], F32, name=f"pss_{g}_{bl}_{h}",
                                 tag="s")
                nc.tensor.matmul(ps_s[:, 0:128], identb_sb[:], trimask_sb[:],
                                 start=True, stop=False)
                nc.tensor.matmul(
                    ps_s[:, 0:128],
                    KT[:, q0:q0 + 128],
                    QT[:, q0:q0 + 128],
                    start=False, stop=True,
                )
                nc.tensor.matmul(ps_s[:, 256:384], identb_sb[:], trimask_sb[:],
                                 start=True, stop=False)
                nc.tensor.matmul(
                    ps_s[:, 256:384],
                    KT[:, q0 + 128:q0 + 256],
                    QT[:, q0 + 128:q0 + 256],
                    start=False, stop=True,
                )
                nc.tensor.matmul(
                    ps_s[:, 128:256],
                    KT[:, q0:q0 + 128],
                    QT[:, q0 + 128:q0 + 256],
                    start=True, stop=True,
                )

                P_sb = pp.tile([128, 384], BF16, name=f"P_{g}_{bl}_{h}",
                               tag="P")
                nc.scalar.activation(P_sb[:], ps_s[:], AF.Exp)
                hs[(bl, h, "P")] = P_sb

            def pv(bl, h):
                P_sb = hs.pop((bl, h, "P"))
                vt0 = V65_sb[:, 2 * bl, 65 * h:65 * h + 65]
                vt1 = V65_sb[:, 2 * bl + 1, 65 * h:65 * h + 65]
                # flipped: O[q, f] with q on partitions; col 64/129 = Z
                ps_o = o_ps.tile([128, 130], F32, name=f"pso_{g}_{bl}_{h}",
                                 tag="o")
                nc.tensor.matmul(ps_o[:, 0:65], P_sb[:, 0:128], vt0,
                                 start=True, stop=True)
                nc.tensor.matmul(ps_o[:, 65:130], P_sb[:, 128:256], vt0,
                                 start=True, stop=False)
                nc.tensor.matmul(ps_o[:, 65:130], P_sb[:, 256:384], vt1,
                                 start=False, stop=True)
                hs[(bl, h, "o")] = ps_o

            def norma(bl, h):
                ps_o = hs.pop((bl, h, "o"))
                i = bl * H + h
                Oq_sb = oe.tile([128, 130], F32, name=f"Oq_{g}_{bl}_{h}",
                                tag="Oq")
                if i % 2 == 0:
                    nc.vector.tensor_copy(Oq_sb[:], ps_o[:])
                else:
                    nc.scalar.copy(Oq_sb[:], ps_o[:])
                hs[(bl, h, "q")] = Oq_sb

            def normb(bl, h):
                Oq_sb = hs.pop((bl, h, "q"))
                # per-partition (per-q) normalize on Pool; overwrites Z cols
                On_sb = on.tile([128, 128], BF16, name=f"On_{g}_{bl}_{h}",
                                tag="On")
                nc.gpsimd.normalize_recip(On_sb[:, 0:64], Oq_sb[:, 0:64],
                                          Oq_sb[:, 64:65])
                nc.gpsimd.normalize_recip(On_sb[:, 64:128], Oq_sb[:, 65:129],
                                          Oq_sb[:, 129:130])
                hs[(bl, h, "n")] = On_sb

            def otr(bl, h):
                On_sb = hs.pop((bl, h, "n"))
                q0 = 256 * bl
                hp, prow = h // 2, 64 * (h % 2)
                ps_t = tr_ps.tile([128, 128], BF16, name=f"ptr_{g}_{bl}_{h}",
                                  tag="tr")
                nc.tensor.transpose(ps_t[:], On_sb[:], identb_sb[:])
                # rows 0:64 = O^T q-tile0, rows 64:128 = O^T q-tile1
                nc.vector.tensor_copy(
                    OT_sb[prow:prow + 64, hp, q0:q0 + 128], ps_t[0:64, :])
                nc.vector.tensor_copy(
                    OT_sb[prow:prow + 64, hp, q0 + 128:q0 + 256],
                    ps_t[64:128, :])

            def proj(tt):
                ps_y = mm_ps.tile([128, 384], F32, name=f"psy_{g}_{tt}",
                                  tag="mm")
                for hp in range(3):
                    nc.tensor.matmul(
                        ps_y[:],
                        OT_sb[:, hp, 128 * tt:128 * (tt + 1)],
                        wpt_sb[:, hp, :],
                        start=(hp == 0),
                        stop=(hp == 2),
                    )
                nc.vector.tensor_tensor(Y_sb[:, tt, :], ps_y[:], beff_sb[:],
                                        op=ALU.add)

            from functools import partial
            heads = [(bl, h) for bl in range(G) for h in range(H)]
            chunks = []
            for i in range(len(heads) + 4):
                wave = []
                if i < len(heads):
                    wave.append(partial(scores, *heads[i]))
                if 1 <= i <= len(heads):
                    wave.append(partial(pv, *heads[i - 1]))
                if 2 <= i <= len(heads) + 1:
                    wave.append(partial(norma, *heads[i - 2]))
                if 3 <= i <= len(heads) + 2:
                    wave.append(partial(normb, *heads[i - 3]))
                if 4 <= i:
                    wave.append(partial(otr, *heads[i - 4]))
                chunks.append(lambda w=wave: [f() for f in w])
            chunks += [partial(proj, tt) for tt in range(NTT)]
            chunks += [lambda: dma(yv[g], Y_sb[:])]
            return chunks

        # software pipeline: interleave gen(g+1) between attention(g) chunks
        for c in gen_chunks(0):
            c()
        for g in range(NG):
            att = att_chunks(g)
            gen = gen_chunks(g + 1) if g + 1 < NG else []
            n = max(len(att), len(gen))
            for i in range(n):
                if i < len(att):
                    att[i]()
                if i < len(gen):
                    gen[i]()


_CACHE = {}


def _build_nc():
    if "nc" in _CACHE:
        return _CACHE["nc"]
    nc = bacc.Bacc("TRN2", target_bir_lowering=False, debug=False,
                   num_devices=N_CORES)
    x_d = nc.dram_tensor("x", [TOK, C], BF16, kind="ExternalInput")
    wat_d = nc.dram_tensor("w_attnT", [C, 3 * C], BF16, kind="ExternalInput")
    wpt_d = nc.dram_tensor("w_projT", [128, 3, C], BF16, kind="ExternalInput")
    bq_d = nc.dram_tensor("bq", [128, 3], F32, kind="ExternalInput")
    bk_d = nc.dram_tensor("bk", [128, 3], F32, kind="ExternalInput")
    beff_d = nc.dram_tensor("beff", [128, C], F32, kind="ExternalInput")
    trimask_d = nc.dram_tensor("trimask", [128, 128], BF16, kind="ExternalInput")
    identb_d = nc.dram_tensor("identb", [128, 128], BF16, kind="ExternalInput")
    y_d = nc.dram_tensor("y", [TOK, C], F32, kind="ExternalOutput")

    with tile.TileContext(nc) as tc:
        _body(tc, x_d, wat_d, wpt_d, bq_d, bk_d, beff_d, trimask_d,
              identb_d, y_d)
    nc.compile()
    _CACHE["nc"] = nc
    return nc


def _host_inputs(x, w_attn, b_attn, w_proj, b_proj):
    """Build the per-core input maps (host-side prep of weights/constants)."""
    bf16 = ml_dtypes.bfloat16
    w_attnT = np.ascontiguousarray(w_attn.T).astype(np.float32)    # [C, 3C]
    w_attnT[:, :C] *= 0.125          # fold score scale into Q weights
    # w_projT per head pair: wpt[p, hp, of] = w_proj[of, 128*hp + p]
    wpt = np.ascontiguousarray(
        w_proj.T.reshape(3, 128, C).transpose(1, 0, 2))
    bq = np.ascontiguousarray((0.125 * b_attn[:C]).reshape(3, 128).T)
    bk = np.ascontiguousarray(b_attn[C:2 * C].reshape(3, 128).T)
    b_eff = np.broadcast_to(b_proj + w_proj @ b_attn[2 * C:], (128, C))

    # shared triangular premask for both diagonal [128,128] quadrants
    p = np.arange(128)[:, None]
    j = np.arange(128)[None, :]
    mask = np.where(p <= j, 0.0, NEGBIG).astype(np.float32)

    ident = np.eye(128, dtype=np.float32)

    common = {
        "w_attnT": w_attnT.astype(bf16),
        "w_projT": wpt.astype(bf16),
        "bq": bq.astype(np.float32),
        "bk": bk.astype(np.float32),
        "beff": np.ascontiguousarray(b_eff).astype(np.float32),
        "trimask": mask.astype(bf16),
        "identb": ident.astype(bf16),
    }
    xs = x.reshape(N_CORES, TOK, C)
    in_maps = []
    for c in range(N_CORES):
        m = dict(common)
        m["x"] = np.ascontiguousarray(xs[c]).astype(bf16)
        in_maps.append(m)
    return in_maps


def kernel(x, w_attn, b_attn, w_proj, b_proj):
    from concourse.bass_utils import run_bass_kernel_spmd

    x = np.asarray(x, dtype=np.float32)
    w_attn = np.asarray(w_attn, dtype=np.float32)
    b_attn = np.asarray(b_attn, dtype=np.float32)
    w_proj = np.asarray(w_proj, dtype=np.float32)
    b_proj = np.asarray(b_proj, dtype=np.float32)

    nc = _build_nc()
    in_maps = _host_inputs(x, w_attn, b_attn, w_proj, b_proj)
    res = run_bass_kernel_spmd(nc, in_maps, core_ids=list(range(N_CORES)))
    y = np.stack([res.results[c]["y"] for c in range(N_CORES)])
    return y.reshape(B, T, C)



# revision 2
# speedup vs baseline: 1.0064x; 1.0064x over previous
"""Causal self-attention Trainium2 kernel.

Full inputs -> full outputs. Data-parallel over batch across 8 NeuronCores
(16 batches per core), no collectives.

Per-core design (bf16 matmul operands, fp32 PSUM accumulate):
  - x is pre-transposed ON HOST to XT [C, tok] bf16 (DMA'd per group), so
    the kernel never spends PE cycles transposing activations.
  - Q^T/K^T [feature, tok] bf16: lhsT = w_attn^T chunk, rhs = XT; the 1/8
    score scale is folded into the Q weights/bias on the host; bias applied
    on eviction, alternating ACT (with bias) / DVE (tensor_scalar_add) to
    balance engine load.
  - V [tok, feature] bf16 with an interleaved ones column per head so row 64
    of the PV output is Z = sum_k P.
  - Scores per (batch, head) are computed transposed and PACKED [k=128, 384]
    with only TWO matmuls (no mask matmuls): cols 0:256 = k-tile0 x
    (q 0..255) in one N=256 matmul (shared lhsT), cols 256:384 = k-tile1 x
    (q 128..255). The fully-masked (k-tile1, q<128) quadrant is never
    computed. One ACT exp eviction -> P bf16, then the two diagonal
    quadrants are multiplied by a 0/1 triangular mask on DVE (cheaper than
    the old -1e30 identity-matmul pre-bias, which cost 2 PE matmuls/head).
  - PV: 3 N=65 matmuls -> O~ [128q, 130] (cols 64/129 = Z per q-tile).
  - Normalize per-partition (per-q) on Pool (normalize_recip), transpose
    On on PE, copy into the pair-stacked OT tile (head parity picks
    partitions 0:64 / 64:128), so projection runs with K=128 lhsT tiles.
  - Projection: 3 K=128 accumulating matmuls per 128-token tile; bias added
    on the DVE eviction (tensor_tensor add with a broadcast bias tile);
    one batched output DMA per group.
"""

import numpy as np
import ml_dtypes

import concourse.bass as bass
import concourse.bacc as bacc
import concourse.mybir as mybir
import concourse.tile as tile

N_CORES = 8
B, T, C = 128, 256, 384
H, HD = 6, 64
NB = B // N_CORES          # batches per core
TOK = NB * T               # tokens per core
G = 2                      # batches per group
NG = NB // G               # groups per core
GT = G * T                 # tokens per group (512)
NTT = GT // 128            # 128-token tiles per group (4)
F32 = mybir.dt.float32
BF16 = mybir.dt.bfloat16
AF = mybir.ActivationFunctionType
ALU = mybir.AluOpType


def _body(tc, xt_d, wat_d, wpt_d, bq_d, bk_d, beff_d, trim_d,
          identb_d, y_d):
    nc = tc.nc
    from contextlib import ExitStack

    ctx = ExitStack()
    with ctx:
        const = ctx.enter_context(tc.tile_pool(name="const", bufs=1))
        xt = ctx.enter_context(tc.tile_pool(name="xt", bufs=2))
        qkt = ctx.enter_context(tc.tile_pool(name="qkt", bufs=2))
        v65 = ctx.enter_context(tc.tile_pool(name="v65", bufs=2))
        pp = ctx.enter_context(tc.tile_pool(name="pp", bufs=6))
        oe = ctx.enter_context(tc.tile_pool(name="oe", bufs=6))
        on = ctx.enter_context(tc.tile_pool(name="on", bufs=4))
        ot = ctx.enter_context(tc.tile_pool(name="ot", bufs=2))
        yb = ctx.enter_context(tc.tile_pool(name="yb", bufs=2))
        mm_ps = ctx.enter_context(tc.tile_pool(name="mm_ps", bufs=2, space="PSUM"))
        s_ps = ctx.enter_context(tc.tile_pool(name="s_ps", bufs=2, space="PSUM"))
        o_ps = ctx.enter_context(tc.tile_pool(name="o_ps", bufs=2, space="PSUM"))
        tr_ps = ctx.enter_context(tc.tile_pool(name="tr_ps", bufs=2, space="PSUM"))

        dma = nc.sync.dma_start

        wat_sb = const.tile([128, 3, 3 * C], BF16, name="wat_sb")
        wpt_sb = const.tile([128, 3, C], BF16, name="wpt_sb")
        bq_sb = const.tile([128, 3], F32, name="bq_sb")
        bk_sb = const.tile([128, 3], F32, name="bk_sb")
        beff_sb = const.tile([128, C], F32, name="beff_sb")
        trim_sb = const.tile([128, 128], BF16, name="trim_sb")
        identb_sb = const.tile([128, 128], BF16, name="identb_sb")

        dma(wat_sb[:], wat_d.ap().rearrange("(ct p) f -> p ct f", p=128))
        dma(wpt_sb[:], wpt_d.ap())
        dma(bq_sb[:], bq_d.ap())
        dma(bk_sb[:], bk_d.ap())
        dma(beff_sb[:], beff_d.ap())
        dma(trim_sb[:], trim_d.ap())
        dma(identb_sb[:], identb_d.ap())

        xtv = xt_d.ap().rearrange("ct p (g t) -> g p ct t", t=GT)
        yv = y_d.ap().rearrange("(g tt p) c -> g p tt c", tt=NTT, p=128)

        # Tiles live across the gen(g+1) / attention(g) software pipeline.
        st = {}

        def gen_chunks(g):
            """QKV generation for group g as a list of emit-closures."""
            XT_sb = xt.tile([128, 3, GT], BF16, name=f"XT_{g}", tag="XT")
            QKT_sb = qkt.tile([128, 6, GT], BF16, name=f"QKT_{g}", tag="QKT")
            V65_sb = v65.tile([128, NTT, H * 65], BF16, name=f"V65_{g}",
                              tag="V65")
            st[g] = (QKT_sb, V65_sb)
            chunks = [lambda: dma(XT_sb[:], xtv[g])]

            def qkgen(ft):
                ps_qk = mm_ps.tile([128, 512], F32, name=f"psqk_{g}_{ft}",
                                   tag="mm")
                for ct in range(3):
                    nc.tensor.matmul(
                        ps_qk[:],
                        wat_sb[:, ct, 128 * ft:128 * (ft + 1)],
                        XT_sb[:, ct, :],
                        start=(ct == 0),
                        stop=(ct == 2),
                    )
                bias = bq_sb[:, ft:ft + 1] if ft < 3 else bk_sb[:, ft - 3:ft - 2]
                if ft % 2 == 0:
                    nc.scalar.activation(QKT_sb[:, ft, :], ps_qk[:],
                                         AF.Identity, bias=bias)
                else:
                    nc.vector.tensor_scalar_add(QKT_sb[:, ft, :], ps_qk[:],
                                                bias)

            def vgen(tt):
                ps_v = mm_ps.tile([128, 384], F32, name=f"psv_{g}_{tt}",
                                  tag="mm")
                for ct in range(3):
                    nc.tensor.matmul(
                        ps_v[:],
                        XT_sb[:, ct, 128 * tt:128 * (tt + 1)],
                        wat_sb[:, ct, 2 * C:3 * C],
                        start=(ct == 0),
                        stop=(ct == 2),
                    )
                v_view = V65_sb[:, tt, :].rearrange("p (h w) -> p h w", h=H)
                if tt % 2 == 0:
                    nc.vector.tensor_copy(
                        v_view[:, :, 0:64],
                        ps_v[:].rearrange("p (h w) -> p h w", h=H),
                    )
                else:
                    nc.scalar.copy(
                        v_view[:, :, 0:64],
                        ps_v[:].rearrange("p (h w) -> p h w", h=H),
                    )
                nc.gpsimd.memset(v_view[:, :, 64:65], 1.0)

            from functools import partial
            chunks += [partial(qkgen, ft) for ft in range(6)]
            chunks += [partial(vgen, tt) for tt in range(NTT)]
            return chunks

        def att_chunks(g):
            """Attention + projection for group g as a list of emit-closures.

            The per-head work is stage-split (scores -> mask -> PV ->
            normalize -> transpose) and emitted in waves so the PE never
            sits directly behind an ACT exp or a DVE mask.
            """
            QKT_sb, V65_sb = st[g]
            OT_sb = ot.tile([128, 3, GT], BF16, name=f"OT_{g}", tag="OT")
            Y_sb = yb.tile([128, NTT, C], F32, name=f"Y_{g}", tag="Y")
            hs = {}

            def scores(bl, h):
                q0 = 256 * bl
                ft, r0 = h // 2, 64 * (h % 2)
                KT = QKT_sb[r0:r0 + 64, 3 + ft, :]
                QT = QKT_sb[r0:r0 + 64, ft, :]

                ps_s = s_ps.tile([128, 384], F32, name=f"pss_{g}_{bl}_{h}",
                                 tag="s")
                # cols 0:256 = k-tile0 x (q 0:256), one matmul (shared lhsT)
                nc.tensor.matmul(
                    ps_s[:, 0:256],
                    KT[:, q0:q0 + 128],
                    QT[:, q0:q0 + 256],
                    start=True, stop=True,
                )
                # cols 256:384 = k-tile1 x (q 128:256)
                nc.tensor.matmul(
                    ps_s[:, 256:384],
                    KT[:, q0 + 128:q0 + 256],
                    QT[:, q0 + 128:q0 + 256],
                    start=True, stop=True,
                )

                P_sb = pp.tile([128, 384], BF16, name=f"P_{g}_{bl}_{h}",
                               tag="P")
                nc.scalar.activation(P_sb[:], ps_s[:], AF.Exp)
                hs[(bl, h, "P")] = P_sb

            def maskp(bl, h):
                # zero the upper-triangular (k > q) entries of the two
                # diagonal quadrants with a 0/1 mask multiply
                P_sb = hs[(bl, h, "P")]
                nc.vector.tensor_tensor(P_sb[:, 0:128], P_sb[:, 0:128],
                                        trim_sb[:], op=ALU.mult)
                nc.vector.tensor_tensor(P_sb[:, 256:384], P_sb[:, 256:384],
                                        trim_sb[:], op=ALU.mult)

            def pv(bl, h):
                P_sb = hs.pop((bl, h, "P"))
                vt0 = V65_sb[:, 2 * bl, 65 * h:65 * h + 65]
                vt1 = V65_sb[:, 2 * bl + 1, 65 * h:65 * h + 65]
                # O[q, f] with q on partitions; col 64/129 = Z
                ps_o = o_ps.tile([128, 130], F32, name=f"pso_{g}_{bl}_{h}",
                                 tag="o")
                nc.tensor.matmul(ps_o[:, 0:65], P_sb[:, 0:128], vt0,
                                 start=True, stop=True)
                nc.tensor.matmul(ps_o[:, 65:130], P_sb[:, 128:256], vt0,
                                 start=True, stop=False)
                nc.tensor.matmul(ps_o[:, 65:130], P_sb[:, 256:384], vt1,
                                 start=False, stop=True)
                hs[(bl, h, "o")] = ps_o

            def norma(bl, h):
                ps_o = hs.pop((bl, h, "o"))
                i = bl * H + h
                Oq_sb = oe.tile([128, 130], F32, name=f"Oq_{g}_{bl}_{h}",
                                tag="Oq")
                if i % 2 == 0:
                    nc.vector.tensor_copy(Oq_sb[:], ps_o[:])
                else:
                    nc.scalar.copy(Oq_sb[:], ps_o[:])
                hs[(bl, h, "q")] = Oq_sb

            def normb(bl, h):
                Oq_sb = hs.pop((bl, h, "q"))
                # per-partition (per-q) normalize on Pool; overwrites Z cols
                On_sb = on.tile([128, 128], BF16, name=f"On_{g}_{bl}_{h}",
                                tag="On")
                nc.gpsimd.normalize_recip(On_sb[:, 0:64], Oq_sb[:, 0:64],
                                          Oq_sb[:, 64:65])
                nc.gpsimd.normalize_recip(On_sb[:, 64:128], Oq_sb[:, 65:129],
                                          Oq_sb[:, 129:130])
                hs[(bl, h, "n")] = On_sb

            def otr(bl, h):
                On_sb = hs.pop((bl, h, "n"))
                q0 = 256 * bl
                hp, prow = h // 2, 64 * (h % 2)
                ps_t = tr_ps.tile([128, 128], BF16, name=f"ptr_{g}_{bl}_{h}",
                                  tag="tr")
                nc.tensor.transpose(ps_t[:], On_sb[:], identb_sb[:])
                # rows 0:64 = O^T q-tile0, rows 64:128 = O^T q-tile1
                nc.vector.tensor_copy(
                    OT_sb[prow:prow + 64, hp, q0:q0 + 128], ps_t[0:64, :])
                nc.vector.tensor_copy(
                    OT_sb[prow:prow + 64, hp, q0 + 128:q0 + 256],
                    ps_t[64:128, :])

            def proj(tt):
                ps_y = mm_ps.tile([128, 384], F32, name=f"psy_{g}_{tt}",
                                  tag="mm")
                for hp in range(3):
                    nc.tensor.matmul(
                        ps_y[:],
                        OT_sb[:, hp, 128 * tt:128 * (tt + 1)],
                        wpt_sb[:, hp, :],
                        start=(hp == 0),
                        stop=(hp == 2),
                    )
                nc.vector.tensor_tensor(Y_sb[:, tt, :], ps_y[:], beff_sb[:],
                                        op=ALU.add)

            from functools import partial
            heads = [(bl, h) for bl in range(G) for h in range(H)]
            chunks = []
            for i in range(len(heads) + 5):
                wave = []
                if i < len(heads):
                    wave.append(partial(scores, *heads[i]))
                if 1 <= i <= len(heads):
                    wave.append(partial(maskp, *heads[i - 1]))
                if 2 <= i <= len(heads) + 1:
                    wave.append(partial(pv, *heads[i - 2]))
                if 3 <= i <= len(heads) + 2:
                    wave.append(partial(norma, *heads[i - 3]))
                if 4 <= i <= len(heads) + 3:
                    wave.append(partial(normb, *heads[i - 4]))
                if 5 <= i:
                    wave.append(partial(otr, *heads[i - 5]))
                chunks.append(lambda w=wave: [f() for f in w])
            chunks += [partial(proj, tt) for tt in range(NTT)]
            chunks += [lambda: dma(yv[g], Y_sb[:])]
            return chunks

        # software pipeline: interleave gen(g+1) between attention(g) chunks
        for c in gen_chunks(0):
            c()
        for g in range(NG):
            att = att_chunks(g)
            gen = gen_chunks(g + 1) if g + 1 < NG else []
            n = max(len(att), len(gen))
            for i in range(n):
                if i < len(att):
                    att[i]()
                if i < len(gen):
                    gen[i]()


_CACHE = {}


def _build_nc():
    if "nc" in _CACHE:
        return _CACHE["nc"]
    nc = bacc.Bacc("TRN2", target_bir_lowering=False, debug=False,
                   num_devices=N_CORES)
    xt_d = nc.dram_tensor("xt", [3, 128, TOK], BF16, kind="ExternalInput")
    wat_d = nc.dram_tensor("w_attnT", [C, 3 * C], BF16, kind="ExternalInput")
    wpt_d = nc.dram_tensor("w_projT", [128, 3, C], BF16, kind="ExternalInput")
    bq_d = nc.dram_tensor("bq", [128, 3], F32, kind="ExternalInput")
    bk_d = nc.dram_tensor("bk", [128, 3], F32, kind="ExternalInput")
    beff_d = nc.dram_tensor("beff", [128, C], F32, kind="ExternalInput")
    trim_d = nc.dram_tensor("trim", [128, 128], BF16, kind="ExternalInput")
    identb_d = nc.dram_tensor("identb", [128, 128], BF16, kind="ExternalInput")
    y_d = nc.dram_tensor("y", [TOK, C], F32, kind="ExternalOutput")

    with tile.TileContext(nc) as tc:
        _body(tc, xt_d, wat_d, wpt_d, bq_d, bk_d, beff_d, trim_d,
              identb_d, y_d)
    nc.compile()
    _CACHE["nc"] = nc
    return nc


def _host_inputs(x, w_attn, b_attn, w_proj, b_proj):
    """Build the per-core input maps (host-side prep of weights/constants)."""
    bf16 = ml_dtypes.bfloat16
    w_attnT = np.ascontiguousarray(w_attn.T).astype(np.float32)    # [C, 3C]
    w_attnT[:, :C] *= 0.125          # fold score scale into Q weights
    # w_projT per head pair: wpt[p, hp, of] = w_proj[of, 128*hp + p]
    wpt = np.ascontiguousarray(
        w_proj.T.reshape(3, 128, C).transpose(1, 0, 2))
    bq = np.ascontiguousarray((0.125 * b_attn[:C]).reshape(3, 128).T)
    bk = np.ascontiguousarray(b_attn[C:2 * C].reshape(3, 128).T)
    b_eff = np.broadcast_to(b_proj + w_proj @ b_attn[2 * C:], (128, C))

    # 0/1 lower-triangular (k <= q) mask for the diagonal quadrants
    p = np.arange(128)[:, None]
    j = np.arange(128)[None, :]
    mask = (p <= j).astype(np.float32)

    ident = np.eye(128, dtype=np.float32)

    common = {
        "w_attnT": w_attnT.astype(bf16),
        "w_projT": wpt.astype(bf16),
        "bq": bq.astype(np.float32),
        "bk": bk.astype(np.float32),
        "beff": np.ascontiguousarray(b_eff).astype(np.float32),
        "trim": mask.astype(bf16),
        "identb": ident.astype(bf16),
    }
    # host-side transpose: x [B,T,C] -> per-core XT [3, 128, TOK]
    xs = x.reshape(N_CORES, TOK, C)
    in_maps = []
    for c in range(N_CORES):
        xt_c = np.ascontiguousarray(xs[c].T.reshape(3, 128, TOK))
        m = dict(common)
        m["xt"] = xt_c.astype(bf16)
        in_maps.append(m)
    return in_maps


def kernel(x, w_attn, b_attn, w_proj, b_proj):
    from concourse.bass_utils import run_bass_kernel_spmd

    x = np.asarray(x, dtype=np.float32)
    w_attn = np.asarray(w_attn, dtype=np.float32)
    b_attn = np.asarray(b_attn, dtype=np.float32)
    w_proj = np.asarray(w_proj, dtype=np.float32)
    b_proj = np.asarray(b_proj, dtype=np.float32)

    nc = _build_nc()
    in_maps = _host_inputs(x, w_attn, b_attn, w_proj, b_proj)
    res = run_bass_kernel_spmd(nc, in_maps, core_ids=list(range(N_CORES)))
    y = np.stack([res.results[c]["y"] for c in range(N_CORES)])
    return y.reshape(B, T, C)
